# revision 21
# baseline (speedup 1.0000x reference)
"""Trainium2 Bass kernel for nn_MultiHeadAttention_52192442581071.

Reference math:
    qp  = query @ Wq.T                                   [bs, D]
    kp  = keys @ Wk.T ; vp = values @ Wv.T               [sl, bs, D]
    dot = (qp * kp).reshape(sl,bs,H,64).sum(-1)/8        [sl, bs, H]
    w   = log_softmax(dot, axis=0)
    attn= (w[...,None] * vp.reshape(sl,bs,H,64)).sum(0)  [bs, H, 64]
    out = attn.reshape(bs,-1) @ Wo.T                     [bs, D]

Algebraic restructuring (exact in exact arithmetic):
    r[b,h,:]   = sum_{j in head h} qp[b,j] * Wk[j,:] / 8          (small)
    dot[s,b,h] = keys[s,b,:] . r[b,h,:]                            (matmul)
    z[b,h,:]   = sum_s (dot-lse) * values[s,b,:] = P - lse*V
        P[b,h,:] = sum_s dot[s,b,h] * values[s,b,:]                (matmul)
        V[b,:]   = sum_s values[s,b,:]          (ones-row trick)
    attn[b,h,n] = P[b,h,:] . Wv[h*64+n,:] - lse[b,h]*(V[b,:] . Wv[h*64+n,:])
    out = attn @ Wo.T                                              (small)

Performance notes:
  * HBM traffic is the binding resource; wire formats are chosen per
    error tolerance of each path (gate is rel_err < 2e-2):
      - keys, Wq, Wk, q, rT: fp8 e4m3. These only feed the logits
        `dot`; an absolute logit error of ~0.03 is invisible after the
        log-softmax.  The fp8e4 pair layout enables DoubleRow matmuls
        (2 k-chunks per instruction, 2x PE rate) for the dot matmul.
      - values: fp8 e3m4 (4 mantissa bits, range +-15.5 fits N(0,1)),
        quantized with error-diffusion dithering along the sequence
        axis so column sums are preserved: the dominant output term is
        lse * (sum_s values) @ Wv.T, and dithering keeps that sum
        accurate to one quantization step instead of sqrt(SL) steps.
      - Wv, Wo, dotT tiles: bf16 (these multiply the large attn terms;
        fp8 here fails the error gate - measured in emulation).
    Accumulation stays fp32 in PSUM; softmax stats fp32.
  * Host-side sharding pre-packs layouts (keys transposed to [d, s]
    with paired d-chunks, weights pre-permuted): no on-chip transposes
    of the streamed data.
  * dot is stored unshifted (dot ~ N(0,1) sits where e4m3 is accurate;
    exp(dot) <= e^5.5 fits fp32); any common shift cancels in the
    log-sum-exp correction.
  * P/V accumulate v-STATIONARY: psP[d, c] += sum_s v[s,d]*dotT[s,c].
    The 33-wide moving side is cheap on the PE (measured 15ns vs 85ns
    for the 512-wide moving-v form; ldweights pipelines with compute),
    and the psum IS the transposed z layout with V in column 32 - no
    z/V transposes or staging. dc-outer so psum accumulation groups
    are sequential (one 2KB zero-region per group).
  * The -lse*V correction is applied in attn space (the Wv projection
    is linear), so P projections do not wait for softmax statistics.
  * fp8 DMAs are issued as uint32 bitcasts: the DMA engine moves
    elements, not bytes, so 1-byte elements would halve bandwidth.
  * Cross-body tiles are double-buffered (bufs=2) so body j+1's weight
    DMAs do not head-of-line block behind body j's tail readers.
  * All bulk DMAs ride the single SP queue (measured fastest on HW;
    multi-queue DMA loses ~30% bandwidth on real silicon), the output
    DMA rides the Pool queue so the next invocation's stream is not
    head-of-line blocked, and loop-invariant constants load once.

Sharding: data-parallel over bs. Core i handles batch rows [4i, 4i+4).
No collectives; same program on all 8 cores with different inputs.
"""

import sys

if "/opt/trn_rl_repo" not in sys.path:
    sys.path.insert(0, "/opt/trn_rl_repo")

import numpy as np

import concourse.bass as bass
import concourse.mybir as mybir
import concourse.tile as tile
from concourse import bacc, bass_utils

# Problem constants (hardcoded per contract)
H = 16          # num heads
NHID = 64
D = 1024
SL = 2048
BS = 32
NCORES = 8
B = BS // NCORES  # 4 local batch elements per core

FP32 = mybir.dt.float32
BF16 = mybir.dt.bfloat16
E4 = mybir.dt.float8e4      # e4m3
E3 = mybir.dt.float8e3      # e3m4
NP_BF16 = mybir.dt.np(BF16)
NP_E4 = mybir.dt.np(E4)
NP_E3 = mybir.dt.np(E3)
DR = mybir.MatmulPerfMode.DoubleRow
Exp = mybir.ActivationFunctionType.Exp
Ln = mybir.ActivationFunctionType.Ln
X = mybir.AxisListType.X
U32 = mybir.dt.uint32


def dma32(eng, dst, src):
    """DMA fp8 tiles as uint32: the DMA engine's throughput is per
    element, so 1-byte elements move at half the byte bandwidth. Same
    bytes, 4x fewer elements. Requires contiguous last dim."""
    eng.dma_start(dst.bitcast(U32), src.bitcast(U32))

import os
KP_BUFS = int(os.environ.get("KP_BUFS", "5"))
VP_BUFS = int(os.environ.get("VP_BUFS", "8"))

DC = D // 128     # 8 d-chunks
DC2 = DC // 2     # 4 paired d-chunks (DoubleRow)
JC = D // 128     # 8 j-chunks
JC2 = JC // 2
NSB = int(os.environ.get("NSB", "4"))  # kv slabs per sequence
SBLK = SL // NSB   # s per slab
NST = SBLK // 128  # s-tiles per slab
NBLK = SBLK // 512  # 512-wide dot blocks per slab
LOGIT_SHIFT = 0.0   # dot stored raw: dot ~ N(0,1) sits where e4m3 is
                    # accurate, and exp(dot) <= e^5.5 fits fp32 easily
R17 = 33            # dot rows: 16 heads + pad + ones row 32 (V acc;
                    # engine reads must start at partition 0/32/64/96)


def build_program(loop_n=1, loop_bodies=1):
    nc = bacc.Bacc(
        "TRN2", target_bir_lowering=False, debug=False,
        enable_asserts=False, num_devices=1,
    )
    qT_d = nc.dram_tensor("qT", [128, DC2, 2, B], E4,
                          kind="ExternalInput").ap()
    kT_d = nc.dram_tensor("kT", [B, NSB, 128, DC2, 2, SBLK], E4,
                          kind="ExternalInput").ap()
    v_d = nc.dram_tensor("vv", [B, NSB, 128, NST, D], E4,
                         kind="ExternalInput").ap()
    wqT_d = nc.dram_tensor("wqT", [128, DC2, 2, D], E4,
                           kind="ExternalInput").ap()
    wk_d = nc.dram_tensor("wk", [128, JC2, 2, D], E4,
                          kind="ExternalInput").ap()
    wvT_d = nc.dram_tensor("wvT", [128, DC, D], BF16,
                           kind="ExternalInput").ap()
    woT_d = nc.dram_tensor("woT", [128, JC, D], BF16,
                           kind="ExternalInput").ap()
    out_d = nc.dram_tensor("out", [B, D], FP32, kind="ExternalOutput").ap()
    ident_d = nc.inline_tensor(np.eye(33, dtype=np.float32), "ident").ap()
    # mask[p, jc, h] = 1/8 if head(jc*128+p) == h else 0
    mask_np = np.zeros((128, JC, H), dtype=np.float32)
    for jc in range(JC):
        for p in range(128):
            mask_np[p, jc, (jc * 128 + p) // NHID] = 0.125
    mask_d = nc.inline_tensor(mask_np, "headmask").ap()
    # maskT8[h, jc, p] = 1 if head(jc*128+p) == h else 0 (for lse scatter)
    maskT_np = np.zeros((H, JC, 128), dtype=np.float32)
    for jc in range(JC):
        for p in range(128):
            maskT_np[(jc * 128 + p) // NHID, jc, p] = 1.0
    maskT_d = nc.inline_tensor(maskT_np, "headmaskT").ap()

    from contextlib import ExitStack
    with tile.TileContext(nc) as tc:
        with ExitStack() as ctx:
            # pools hoisted out of the loop: no per-iteration alloc/drain
            pools = dict(
                const=ctx.enter_context(tc.tile_pool(name="const", bufs=1)),
                kp=ctx.enter_context(tc.tile_pool(name="kp", bufs=KP_BUFS)),
                vp=ctx.enter_context(tc.tile_pool(name="vp", bufs=VP_BUFS)),
                d17p=ctx.enter_context(tc.tile_pool(name="d17p", bufs=8)),
                stats=ctx.enter_context(tc.tile_pool(name="stats", bufs=1)),
                pre=ctx.enter_context(tc.tile_pool(name="pre", bufs=1)),
                psum_tr=ctx.enter_context(
                    tc.tile_pool(name="psum_tr", bufs=2, space="PSUM")),
                psum_acc=ctx.enter_context(
                    tc.tile_pool(name="psum_acc", bufs=2, space="PSUM")),
                psum_pv=ctx.enter_context(
                    tc.tile_pool(name="psum_pv", bufs=2, space="PSUM")),
            )
            # loop-invariant constants: loaded once, before the loop, so
            # iteration i+1's DMA queue is not blocked behind iteration
            # i's late readers of these tiles.
            const = pools["const"]
            ident = const.tile([33, 33], FP32, name="ident_sb")
            nc.sync.dma_start(ident[:], ident_d)
            mask_sb = const.tile([128, JC, H], FP32, name="mask_sb")
            nc.sync.dma_start(mask_sb[:], mask_d)
            maskT = const.tile([H, JC, 128], FP32, name="maskT_sb")
            nc.sync.dma_start(maskT[:], maskT_d)
            # dot rows 0..15 rewritten inside the loop; row 16 ones.
            dot_tiles = []
            for i in range(2):
                dot_b = const.tile([R17, SL], FP32, name=f"dot{i}")
                nc.vector.memset(dot_b[:], 0.0)
                nc.vector.memset(dot_b[32:33, :], 1.0)
                dot_tiles.append(dot_b)
            consts = (ident, mask_sb, maskT, dot_tiles)
            if loop_n > 1:
                with tc.For_i(0, loop_n, 1,
                              staggered_reset=os.environ.get("SRESET", "0") == "1"):
                    for _ in range(loop_bodies):
                        _body(tc, pools, consts, out_d, qT_d, kT_d, v_d,
                              wqT_d, wk_d, wvT_d, woT_d)
            else:
                _body(tc, pools, consts, out_d, qT_d, kT_d, v_d, wqT_d, wk_d,
                      wvT_d, woT_d)
    nc.compile()
    return nc


DMA_ONLY = os.environ.get("DMA_ONLY", "0") == "1"
ABL = os.environ.get("ABL", "")  # timing-only ablations: pv1|dot|tail


def _body(tc, pools, consts, out_d, qT_d, kT_d, v_d, wqT_d, wk_d, wvT_d,
          woT_d):
    nc = tc.nc
    ident, mask_sb, maskT, dot_tiles = consts
    if DMA_ONLY:
        kp = pools["kp"]
        vp = pools["vp"]
        pre = pools["pre"]
        const = pools["const"]
        wqT = pre.tile([128, DC2, 2, D], E4, bufs=2, name="wqT_sb")
        wk = pre.tile([128, JC2, 2, D], E4, bufs=2, name="wk_sb")
        dma32(nc.sync, wqT[:], wqT_d)
        dma32(nc.sync, wk[:], wk_d)
        qT = pre.tile([128, DC2, 2, B], E4, bufs=2, name="qT_sb")
        dma32(nc.sync, qT[:], qT_d)
        wvT = const.tile([128, DC, D], BF16, bufs=2, name="wvT")
        woT = const.tile([128, JC, D], BF16, bufs=2, name="woT")
        for b in range(B):
            for sblk in range(NSB):
                kT = kp.tile([128, DC2, 2, SBLK], E4, tag="kT",
                             name=f"kT_{b}_{sblk}")
                dma32(nc.sync, kT[:], kT_d[b, sblk])
                v_t = vp.tile([128, NST, D], E4, tag="v",
                              name=f"v_{b}_{sblk}")
                dma32(nc.sync, v_t[:], v_d[b, sblk])
                if sblk == 0 and b == 1:
                    nc.sync.dma_start(wvT[:], wvT_d)
                if sblk == 0 and b == 2:
                    nc.sync.dma_start(woT[:], woT_d)
        return
    if True:
        const = pools["const"]
        kp = pools["kp"]
        vp = pools["vp"]
        d17p = pools["d17p"]
        stats = pools["stats"]
        pre = pools["pre"]
        psum_tr = pools["psum_tr"]
        psum_acc = pools["psum_acc"]
        psum_pv = pools["psum_pv"]

        # ---- weight/const DMAs ------------------------------------------
        # DMAQ: which queues carry DMAs. Real-HW probe showed a single
        # queue reaches full bandwidth while SWDGE (Pool) DMAs are slow.
        dmaq_mode = os.environ.get("DMAQ", "one")
        qs = {"one": [nc.sync],
              "wact": [nc.sync],
              "two": [nc.sync, nc.scalar],
              "three": [nc.sync, nc.scalar, nc.gpsimd]}[dmaq_mode]
        nq = len(qs)
        # weight queue: in "wact" mode the weights ride the Act queue,
        # freeing the SP queue for the kv stream.
        wq_eng = nc.scalar if dmaq_mode == "wact" else qs[0]
        wqT = pre.tile([128, DC2, 2, D], E4, bufs=2, name="wqT_sb")
        wk = pre.tile([128, JC2, 2, D], E4, bufs=2, name="wk_sb")
        if nq == 1:
            dma32(wq_eng, wqT[:], wqT_d)
            dma32(wq_eng, wk[:], wk_d)
        else:
            dma32(qs[0], wqT[:, 0:2], wqT_d[:, 0:2])
            dma32(qs[nq - 1], wqT[:, 2:4], wqT_d[:, 2:4])
            dma32(qs[(nq - 1) // 2], wk[:, 0:2], wk_d[:, 0:2])
            dma32(qs[0], wk[:, 2:4], wk_d[:, 2:4])
        # (wvT/woT DMAs are issued mid-stream: they are only needed by
        # the tail projections.)
        wvT = const.tile([128, DC, D], BF16, bufs=2, name="wvT")
        woT = const.tile([128, JC, D], BF16, bufs=2, name="woT")

        qT = pre.tile([128, DC2, 2, B], E4, bufs=2, name="qT_sb")
        dma32(nc.sync, qT[:], qT_d)

        # rT[p, dc2, i, b, h] = r[b, h, (dc2*2+i)*128+p]  (incl 1/8 scale)
        rT = const.tile([128, DC2, 2, B, H], E4, bufs=2, name="rT")
        # zTV[p, dc, b, c]: c<16 -> P[b, c, dc*128+p]; c=32 -> V[b, .]
        # (v-stationary P matmuls emit P and V already transposed)
        zTV = const.tile([128, DC, B, R17], BF16, bufs=2, name="zTV")
        nl128 = const.tile([128, JC, B], FP32, bufs=2, name="nl128")
        VWv = const.tile([128, JC, B], FP32, bufs=2, name="VWv")
        prod = const.tile([128, JC, B], FP32, bufs=2, name="prod")
        attnT = const.tile([128, JC, B], BF16, bufs=2, name="attnT")
        out_sb = const.tile([B, D], FP32, bufs=2, name="out_sb")

        # ---- preamble: qp, r --------------------------------------------
        # qpT[p, jc, b] = qp[b, jc*128+p]
        qpT = pre.tile([128, JC, B], FP32, bufs=2, name="qpT")
        for jc in range(JC):
            ps = psum_acc.tile([128, B], FP32, tag="acc", name=f"ps_qp{jc}")
            for dc2 in range(DC2):
                nc.tensor.matmul(
                    ps[:], wqT[:, dc2, :, jc * 128:(jc + 1) * 128],
                    qT[:, dc2], start=(dc2 == 0), stop=(dc2 == DC2 - 1),
                    perf_mode=DR)
            nc.vector.tensor_copy(qpT[:, jc, :], ps[:])

        # Q[p, jc, b, h] = qp[b, jc*128+p]/8 if head(jc*128+p)==h else 0
        Q = pre.tile([128, JC, B, H], E4, bufs=2, name="Q")
        nc.vector.tensor_tensor(
            Q[:],
            qpT[:, :, :, None].to_broadcast((128, JC, B, H)),
            mask_sb[:, :, None, :].to_broadcast((128, JC, B, H)),
            mybir.AluOpType.mult)

        # rT[d, (b,h)] = sum_j Wk[j, d] * Q[j, (b,h)]   (DoubleRow: 2 jc
        # chunks per instruction)
        for dc in range(DC):
            ps = psum_acc.tile([128, B * H], FP32, tag="acc", name=f"ps_r{dc}")
            for jc2 in range(JC2):
                nc.tensor.matmul(
                    ps[:], wk[:, jc2, :, dc * 128:(dc + 1) * 128],
                    Q[:, 2 * jc2:2 * jc2 + 2, :, :],
                    start=(jc2 == 0), stop=(jc2 == JC2 - 1), perf_mode=DR)
            nc.vector.tensor_copy(rT[:, dc // 2, dc % 2, :, :], ps[:])

        # ---- main loop: stream keys/values ------------------------------
        S_all = stats.tile([16, B], FP32, tag="S", bufs=2, name="S_all")
        for b in range(B):
            dot_b = dot_tiles[b % 2]
            v_tiles = []
            d17_tiles = []
            scratch = stats.tile([16, 512], FP32, tag="scratch", bufs=2,
                                 name=f"scr{b}")
            S_parts = stats.tile([16, 4], FP32, tag="Sp", bufs=2,
                                 name=f"Sp{b}")
            for sblk in range(NSB):
                # kv slabs round-robin over the DMA queues in use
                i = NSB * b + sblk
                kT = kp.tile([128, DC2, 2, SBLK], E4, tag="kT",
                             name=f"kT_{b}_{sblk}")
                dma32(qs[i % nq], kT[:], kT_d[b, sblk])
                v_t = vp.tile([128, NST, D], E4, tag="v",
                              name=f"v_{b}_{sblk}")
                dma32(qs[(i + 1) % nq], v_t[:], v_d[b, sblk])
                v_tiles.append(v_t)
                # wvT/woT are only read in the tail and double-buffered,
                # so they ride a side DGE queue in parallel with the kv
                # stream, shrinking the critical SP-queue bytes.
                wvq = {"pool": nc.gpsimd, "act": nc.scalar,
                       "sp": qs[0]}[os.environ.get("WVQ", "pool")]
                if sblk == 0 and b == 0:
                    wvq.dma_start(wvT[:], wvT_d)
                if sblk == 1 and b == 0:
                    wvq.dma_start(woT[:], woT_d)
                # dot[h, s'] (shifted by LOGIT_SHIFT) for this slab
                for blk in range(NBLK if ABL != "dot" else 0):
                    s0 = sblk * SBLK + blk * 512
                    ps_dot = psum_acc.tile([16, 512], FP32, tag="acc",
                                           name=f"ps_dot{b}_{sblk}_{blk}")
                    for dc2 in range(DC2):
                        nc.tensor.matmul(
                            ps_dot[:], rT[:, dc2, :, b, :],
                            kT[:, dc2, :, blk * 512:(blk + 1) * 512],
                            start=(dc2 == 0), stop=(dc2 == DC2 - 1),
                            perf_mode=DR)
                    nc.vector.tensor_scalar_add(
                        dot_b[0:16, s0:s0 + 512], ps_dot[:], LOGIT_SHIFT)
                    # partial softmax denominator for this 512-block:
                    # dot_b is dot-4 (|dot| <~ 6) so exp cannot overflow
                    # and no running-max subtraction is needed.
                    nc.scalar.activation(
                        scratch[:, 0:512], dot_b[0:16, s0:s0 + 512], Exp,
                        bias=0.0, scale=1.0,
                        accum_out=S_parts[:, sblk * NBLK + blk:
                                          sblk * NBLK + blk + 1])
                # dotT tiles (4 transposes -> one drain); then
                # v-stationary P/V accumulation: psP[dc][d, c] +=
                # sum_s v[s, d] * dotT[s, c] - the 33-wide moving side
                # rides the PE cheaply, v loads on the ldweights path,
                # and the psum IS the transposed z layout (V in col 32).
                ps_t = psum_tr.tile([128, NST, R17], FP32, tag="tr",
                                    name=f"ps_dt{b}_{sblk}")
                for st in range(NST):
                    cols = slice(sblk * SBLK + st * 128,
                                 sblk * SBLK + (st + 1) * 128)
                    nc.tensor.transpose(ps_t[:, st, :], dot_b[:, cols],
                                        ident[:])
                d17 = d17p.tile([128, NST, R17], E4, tag="d17",
                                name=f"d17_{b}_{sblk}")
                nc.vector.tensor_copy(d17[:], ps_t[:])
                d17_tiles.append(d17)
            # ---- per-b: finish softmax denominator ----------------------
            nc.vector.reduce_sum(S_all[:, b:b + 1], S_parts[:], axis=X)
            # ---- per-b: v-stationary P/V accumulation ------------------
            # psP[d, c] = sum_s v[s, d] * dotT[s, c]: the 33-wide moving
            # side is cheap on the PE, v rides the (pipelined) ldweights
            # path, and the result IS the transposed z layout with V in
            # column 32. dc-outer so accumulation groups are sequential
            # (one psum bank zero-region per group).
            NSP = NSB * NST // 2
            for dc in range(DC):
                psP = psum_pv.tile([128, R17], FP32, tag="pv",
                                   name=f"pv_{b}_{dc}")
                for sp in range(NSP):
                    sts = slice((2 * sp) % NST, (2 * sp) % NST + 2)
                    nc.tensor.matmul(
                        psP[:],
                        v_tiles[sp // 2][:, sts, dc * 128:(dc + 1) * 128],
                        d17_tiles[sp // 2][:, sts, :],
                        start=(sp == 0), stop=(sp == NSP - 1),
                        perf_mode=DR)
                nc.vector.tensor_copy(zTV[:, dc, b, :], psP[:])

        # ---- tail ------------------------------------------------------
        # lse: one Ln pass; scatter -lse[b,h] to the attnT row layout.
        lnS = stats.tile([16, B], FP32, tag="lnS", name="lnS")
        nc.scalar.activation(lnS[:], S_all[:], Ln)
        neg_lse = stats.tile([16, B], FP32, tag="neg_lse", name="nlse")
        nc.vector.tensor_scalar_mul(neg_lse[:], lnS[:], -1.0)
        for jc in range(JC if ABL != "tail" else 0):
            ps_n = psum_tr.tile([128, B], FP32, tag="tr", name=f"ps_nl{jc}")
            nc.tensor.matmul(ps_n[:], maskT[:, jc, :], neg_lse[:],
                             start=True, stop=True)
            nc.vector.tensor_copy(nl128[:, jc, :], ps_n[:])
        # VWv[p, jc, b] = V[b, :] . Wv[jc*128+p, :]
        for jc in range(JC if ABL != "tail" else 0):
            ps_v = psum_acc.tile([128, B], FP32, tag="acc", name=f"ps_vw{jc}")
            for dc in range(DC):
                nc.tensor.matmul(
                    ps_v[:], wvT[:, dc, jc * 128:(jc + 1) * 128],
                    zTV[:, dc, :, 32], start=(dc == 0), stop=(dc == DC - 1))
            nc.vector.tensor_copy(VWv[:, jc, :], ps_v[:])
        # prod = (-lse) * VWv, then attn = P.Wv + prod fused into the
        # PSUM-drain copies of the P.Wv matmuls. The out-projection
        # accumulation is interleaved per jc so PE overlaps it with the
        # remaining attn groups instead of running two serial phases.
        nc.vector.tensor_tensor(prod[:], nl128[:], VWv[:],
                                mybir.AluOpType.mult)
        ps_o0 = psum_tr.tile([B, 512], FP32, tag="tr", name="ps_o0")
        ps_o1 = psum_tr.tile([B, 512], FP32, tag="tr", name="ps_o1")
        for jc in range(JC if ABL != "tail" else 0):
            ps_a = psum_acc.tile([128, B, 2], FP32, tag="acc",
                                 name=f"ps_a{jc}")
            for dc in range(DC):
                nc.tensor.matmul(
                    ps_a[:], wvT[:, dc, jc * 128:(jc + 1) * 128],
                    zTV[:, dc, :, 2 * jc:2 * jc + 2],
                    start=(dc == 0), stop=(dc == DC - 1))
            nc.vector.tensor_tensor(attnT[0:64, jc, :], ps_a[0:64, :, 0],
                                    prod[0:64, jc, :],
                                    mybir.AluOpType.add)
            nc.vector.tensor_tensor(attnT[64:128, jc, :], ps_a[64:128, :, 1],
                                    prod[64:128, jc, :],
                                    mybir.AluOpType.add)
            nc.tensor.matmul(ps_o0[:], attnT[:, jc, :], woT[:, jc, 0:512],
                             start=(jc == 0), stop=(jc == JC - 1))
            nc.tensor.matmul(ps_o1[:], attnT[:, jc, :], woT[:, jc, 512:1024],
                             start=(jc == 0), stop=(jc == JC - 1))
        nc.vector.tensor_copy(out_sb[:, 0:512], ps_o0[:])
        nc.gpsimd.dma_start(out_d[:, 0:512], out_sb[:, 0:512])
        nc.vector.tensor_copy(out_sb[:, 512:1024], ps_o1[:])
        nc.gpsimd.dma_start(out_d[:, 512:1024], out_sb[:, 512:1024])


_NC_CACHE = {}


def get_program():
    if "nc" not in _NC_CACHE:
        _NC_CACHE["nc"] = build_program()
    return _NC_CACHE["nc"]


def _dither_quant(x, npdt):
    """Error-diffusion quantization along axis 0 (preserves column sums
    to within one quantization step)."""
    x = np.asarray(x, dtype=np.float32)
    out = np.empty(x.shape, dtype=npdt)
    e = np.zeros(x.shape[1:], dtype=np.float32)
    for s in range(x.shape[0]):
        t = x[s] + e
        q = t.astype(npdt)
        out[s] = q
        e = t - q.astype(np.float32)
    return out


def make_in_maps(query, keys, values, Wq, Wk, Wv, Wo):
    """Host-side shard + layout packing (permutation / dtype cast)."""
    keys = np.asarray(keys, dtype=np.float32).astype(NP_E4)
    values = _dither_quant(values, NP_E4)

    def permW(W, npdt):  # [p, dc, j] = W[j, dc*128+p]
        W = np.asarray(W, dtype=np.float32).astype(npdt)
        return np.ascontiguousarray(
            W.T.reshape(DC, 128, D).transpose(1, 0, 2))

    def permW2(W, npdt):  # [p, dc2, i, j] = W[j, (dc2*2+i)*128+p]
        W = np.asarray(W, dtype=np.float32).astype(npdt)
        return np.ascontiguousarray(
            W.T.reshape(DC2, 2, 128, D).transpose(2, 0, 1, 3))

    def natW2(W, npdt):  # [p, jc2, i, d] = W[(jc2*2+i)*128+p, d]
        W = np.asarray(W, dtype=np.float32).astype(npdt)
        return np.ascontiguousarray(
            W.reshape(JC2, 2, 128, D).transpose(2, 0, 1, 3))

    wqT = permW2(Wq, NP_E4)
    wk = natW2(Wk, NP_E4)
    wvT = permW(Wv, NP_BF16)
    woT = permW(Wo, NP_BF16)

    # kT[b, sblk, p, dc2, i, s'] = keys[sblk*SBLK+s', b, (dc2*2+i)*128+p]
    kT_all = keys.transpose(1, 2, 0).reshape(BS, DC2, 2, 128, NSB, SBLK)
    kT_all = kT_all.transpose(0, 4, 3, 1, 2, 5)
    # vv[b, sblk, p, st, d] = values[sblk*SBLK+st*128+p, b, d]
    v_all = values.reshape(NSB, NST, 128, BS, D).transpose(3, 0, 2, 1, 4)

    query = np.asarray(query, dtype=np.float32).astype(NP_E4)
    in_maps = []
    for i in range(NCORES):
        sl = slice(B * i, B * (i + 1))
        qT = np.ascontiguousarray(
            query[sl].T.reshape(DC2, 2, 128, B).transpose(2, 0, 1, 3))
        in_maps.append({
            "qT": qT,
            "kT": np.ascontiguousarray(kT_all[sl]),
            "vv": np.ascontiguousarray(v_all[sl]),
            "wqT": wqT, "wk": wk, "wvT": wvT, "woT": woT,
        })
    return in_maps


def kernel(query, keys, values, Wq, Wk, Wv, Wo):
    nc = get_program()
    in_maps = make_in_maps(query, keys, values, Wq, Wk, Wv, Wo)
    res = bass_utils.run_bass_kernel_spmd(nc, in_maps, core_ids=list(range(NCORES)))
    return np.concatenate(
        [res.results[i]["out"] for i in range(NCORES)], axis=0)


# revision 24
# speedup vs baseline: 1.8122x; 1.8122x over previous
"""Trainium2 Bass kernel for nn_MultiHeadAttention_52192442581071.

Reference math:
    qp  = query @ Wq.T                                   [bs, D]
    kp  = keys @ Wk.T ; vp = values @ Wv.T               [sl, bs, D]
    dot = (qp * kp).reshape(sl,bs,H,64).sum(-1)/8        [sl, bs, H]
    w   = log_softmax(dot, axis=0)
    attn= (w[...,None] * vp.reshape(sl,bs,H,64)).sum(0)  [bs, H, 64]
    out = attn.reshape(bs,-1) @ Wo.T                     [bs, D]

Algebraic restructuring (exact in exact arithmetic):
    r[b,h,:]   = sum_{j in head h} qp[b,j] * Wk[j,:] / 8          (small)
    dot[s,b,h] = keys[s,b,:] . r[b,h,:]                            (matmul)
    z[b,h,:]   = sum_s (dot-lse) * values[s,b,:] = P - lse*V
        P[b,h,:] = sum_s dot[s,b,h] * values[s,b,:]                (matmul)
        V[b,:]   = sum_s values[s,b,:]          (ones-row trick)
    attn[b,h,n] = P[b,h,:] . Wv[h*64+n,:] - lse[b,h]*(V[b,:] . Wv[h*64+n,:])
    out = attn @ Wo.T                                              (small)

Performance notes:
  * HBM traffic is the binding resource; wire formats are chosen per
    error tolerance of each path (gate is rel_err < 2e-2):
      - keys, Wq, Wk, q, rT: fp8 e4m3. These only feed the logits
        `dot`; an absolute logit error of ~0.03 is invisible after the
        log-softmax.  The fp8e4 pair layout enables DoubleRow matmuls
        (2 k-chunks per instruction, 2x PE rate) for the dot matmul.
      - values: fp8 e3m4 (4 mantissa bits, range +-15.5 fits N(0,1)),
        quantized with error-diffusion dithering along the sequence
        axis so column sums are preserved: the dominant output term is
        lse * (sum_s values) @ Wv.T, and dithering keeps that sum
        accurate to one quantization step instead of sqrt(SL) steps.
      - Wv, Wo, dotT tiles: bf16 (these multiply the large attn terms;
        fp8 here fails the error gate - measured in emulation).
    Accumulation stays fp32 in PSUM; softmax stats fp32.
  * Host-side sharding pre-packs layouts (keys transposed to [d, s]
    with paired d-chunks, weights pre-permuted): no on-chip transposes
    of the streamed data.
  * dot is stored unshifted (dot ~ N(0,1) sits where e4m3 is accurate;
    exp(dot) <= e^5.5 fits fp32); any common shift cancels in the
    log-sum-exp correction.
  * P/V accumulate v-STATIONARY: psP[d, c] += sum_s v[s,d]*dotT[s,c].
    The 33-wide moving side is cheap on the PE (measured 15ns vs 85ns
    for the 512-wide moving-v form; ldweights pipelines with compute),
    and the psum IS the transposed z layout with V in column 32 - no
    z/V transposes or staging. dc-outer so psum accumulation groups
    are sequential (one 2KB zero-region per group).
  * The -lse*V correction is applied in attn space (the Wv projection
    is linear), so P projections do not wait for softmax statistics.
  * fp8 DMAs are issued as uint32 bitcasts: the DMA engine moves
    elements, not bytes, so 1-byte elements would halve bandwidth.
  * Cross-body tiles are double-buffered (bufs=2) so body j+1's weight
    DMAs do not head-of-line block behind body j's tail readers.
  * All bulk DMAs ride the single SP queue (measured fastest on HW;
    multi-queue DMA loses ~30% bandwidth on real silicon), the output
    DMA rides the Pool queue so the next invocation's stream is not
    head-of-line blocked, and loop-invariant constants load once.

Sharding: data-parallel over bs. Core i handles batch rows [4i, 4i+4).
No collectives; same program on all 8 cores with different inputs.
"""

import sys

if "/opt/trn_rl_repo" not in sys.path:
    sys.path.insert(0, "/opt/trn_rl_repo")

import numpy as np

import concourse.bass as bass
import concourse.mybir as mybir
import concourse.tile as tile
from concourse import bacc, bass_utils

# Problem constants (hardcoded per contract)
H = 16          # num heads
NHID = 64
D = 1024
SL = 2048
BS = 32
NCORES = 8
B = BS // NCORES  # 4 local batch elements per core

FP32 = mybir.dt.float32
BF16 = mybir.dt.bfloat16
E4 = mybir.dt.float8e4      # e4m3
E3 = mybir.dt.float8e3      # e3m4
NP_BF16 = mybir.dt.np(BF16)
NP_E4 = mybir.dt.np(E4)
NP_E3 = mybir.dt.np(E3)
DR = mybir.MatmulPerfMode.DoubleRow
Exp = mybir.ActivationFunctionType.Exp
Ln = mybir.ActivationFunctionType.Ln
X = mybir.AxisListType.X
U32 = mybir.dt.uint32


def dma32(eng, dst, src):
    """DMA fp8 tiles as uint32: the DMA engine's throughput is per
    element, so 1-byte elements move at half the byte bandwidth. Same
    bytes, 4x fewer elements. Requires contiguous last dim."""
    eng.dma_start(dst.bitcast(U32), src.bitcast(U32))

import os
KP_BUFS = int(os.environ.get("KP_BUFS", "5"))
VP_BUFS = int(os.environ.get("VP_BUFS", "12"))

DC = D // 128     # 8 d-chunks
DC2 = DC // 2     # 4 paired d-chunks (DoubleRow)
JC = D // 128     # 8 j-chunks
JC2 = JC // 2
NSB = int(os.environ.get("NSB", "4"))  # kv slabs per sequence
SBLK = SL // NSB   # s per slab
NST = SBLK // 128  # s-tiles per slab
NBLK = SBLK // 512  # 512-wide dot blocks per slab
LOGIT_SHIFT = 0.0   # dot stored raw: dot ~ N(0,1) sits where e4m3 is
                    # accurate, and exp(dot) <= e^5.5 fits fp32 easily
R17 = 33            # dot rows: 16 heads + pad + ones row 32 (V acc;
                    # engine reads must start at partition 0/32/64/96)


def build_program(loop_n=1, loop_bodies=1):
    nc = bacc.Bacc(
        "TRN2", target_bir_lowering=False, debug=False,
        enable_asserts=False, num_devices=1,
    )
    qT_d = nc.dram_tensor("qT", [128, DC2, 2, B], E4,
                          kind="ExternalInput").ap()
    kT_d = nc.dram_tensor("kT", [B, NSB, 128, DC2, 2, SBLK], E4,
                          kind="ExternalInput").ap()
    v_d = nc.dram_tensor("vv", [B, NSB, 128, NST, D], E4,
                         kind="ExternalInput").ap()
    wqT_d = nc.dram_tensor("wqT", [128, DC2, 2, D], E4,
                           kind="ExternalInput").ap()
    wk_d = nc.dram_tensor("wk", [128, JC2, 2, D], E4,
                          kind="ExternalInput").ap()
    wvT_d = nc.dram_tensor("wvT", [128, DC, D], BF16,
                           kind="ExternalInput").ap()
    woT_d = nc.dram_tensor("woT", [128, JC, D], BF16,
                           kind="ExternalInput").ap()
    out_d = nc.dram_tensor("out", [B, D], FP32, kind="ExternalOutput").ap()
    ident_d = nc.inline_tensor(np.eye(33, dtype=np.float32), "ident").ap()
    # mask[p, jc, h] = 1/8 if head(jc*128+p) == h else 0
    mask_np = np.zeros((128, JC, H), dtype=np.float32)
    for jc in range(JC):
        for p in range(128):
            mask_np[p, jc, (jc * 128 + p) // NHID] = 0.125
    mask_d = nc.inline_tensor(mask_np, "headmask").ap()
    # maskT8[h, jc, p] = 1 if head(jc*128+p) == h else 0 (for lse scatter)
    maskT_np = np.zeros((H, JC, 128), dtype=np.float32)
    for jc in range(JC):
        for p in range(128):
            maskT_np[(jc * 128 + p) // NHID, jc, p] = 1.0
    maskT_d = nc.inline_tensor(maskT_np, "headmaskT").ap()

    from contextlib import ExitStack
    with tile.TileContext(nc) as tc:
        with ExitStack() as ctx:
            # pools hoisted out of the loop: no per-iteration alloc/drain
            pools = dict(
                const=ctx.enter_context(tc.tile_pool(name="const", bufs=1)),
                kp=ctx.enter_context(tc.tile_pool(name="kp", bufs=KP_BUFS)),
                vp=ctx.enter_context(tc.tile_pool(name="vp", bufs=VP_BUFS)),
                d17p=ctx.enter_context(tc.tile_pool(name="d17p", bufs=10)),
                stats=ctx.enter_context(tc.tile_pool(name="stats", bufs=1)),
                pre=ctx.enter_context(tc.tile_pool(name="pre", bufs=1)),
                psum_tr=ctx.enter_context(
                    tc.tile_pool(name="psum_tr", bufs=2, space="PSUM")),
                psum_acc=ctx.enter_context(
                    tc.tile_pool(name="psum_acc", bufs=2, space="PSUM")),
                psum_pv=ctx.enter_context(
                    tc.tile_pool(name="psum_pv", bufs=2, space="PSUM")),
            )
            # loop-invariant constants: loaded once, before the loop, so
            # iteration i+1's DMA queue is not blocked behind iteration
            # i's late readers of these tiles.
            const = pools["const"]
            ident = const.tile([33, 33], FP32, name="ident_sb")
            nc.sync.dma_start(ident[:], ident_d)
            mask_sb = const.tile([128, JC, H], FP32, name="mask_sb")
            nc.sync.dma_start(mask_sb[:], mask_d)
            maskT = const.tile([H, JC, 128], FP32, name="maskT_sb")
            nc.sync.dma_start(maskT[:], maskT_d)
            # dot rows 0..15 rewritten inside the loop; row 16 ones.
            dot_tiles = []
            for i in range(2):
                dot_b = const.tile([R17, SL], FP32, name=f"dot{i}")
                nc.vector.memset(dot_b[:], 0.0)
                nc.vector.memset(dot_b[32:33, :], 1.0)
                dot_tiles.append(dot_b)
            consts = (ident, mask_sb, maskT, dot_tiles)
            if loop_n > 1:
                with tc.For_i(0, loop_n, 1,
                              staggered_reset=os.environ.get("SRESET", "0") == "1"):
                    for _ in range(loop_bodies):
                        _body(tc, pools, consts, out_d, qT_d, kT_d, v_d,
                              wqT_d, wk_d, wvT_d, woT_d)
            else:
                _body(tc, pools, consts, out_d, qT_d, kT_d, v_d, wqT_d, wk_d,
                      wvT_d, woT_d)
    nc.compile()
    return nc


DMA_ONLY = os.environ.get("DMA_ONLY", "0") == "1"
ABL = os.environ.get("ABL", "")  # timing-only ablations: pv1|dot|tail


def _body(tc, pools, consts, out_d, qT_d, kT_d, v_d, wqT_d, wk_d, wvT_d,
          woT_d):
    nc = tc.nc
    ident, mask_sb, maskT, dot_tiles = consts
    if DMA_ONLY:
        kp = pools["kp"]
        vp = pools["vp"]
        pre = pools["pre"]
        const = pools["const"]
        wqT = pre.tile([128, DC2, 2, D], E4, bufs=2, name="wqT_sb")
        wk = pre.tile([128, JC2, 2, D], E4, bufs=2, name="wk_sb")
        dma32(nc.sync, wqT[:], wqT_d)
        dma32(nc.sync, wk[:], wk_d)
        qT = pre.tile([128, DC2, 2, B], E4, bufs=2, name="qT_sb")
        dma32(nc.sync, qT[:], qT_d)
        wvT = const.tile([128, DC, D], BF16, bufs=2, name="wvT")
        woT = const.tile([128, JC, D], BF16, bufs=2, name="woT")
        for b in range(B):
            for sblk in range(NSB):
                kT = kp.tile([128, DC2, 2, SBLK], E4, tag="kT",
                             name=f"kT_{b}_{sblk}")
                dma32(nc.sync, kT[:], kT_d[b, sblk])
                v_t = vp.tile([128, NST, D], E4, tag="v",
                              name=f"v_{b}_{sblk}")
                dma32(nc.sync, v_t[:], v_d[b, sblk])
                if sblk == 0 and b == 1:
                    nc.sync.dma_start(wvT[:], wvT_d)
                if sblk == 0 and b == 2:
                    nc.sync.dma_start(woT[:], woT_d)
        return
    if True:
        const = pools["const"]
        kp = pools["kp"]
        vp = pools["vp"]
        d17p = pools["d17p"]
        stats = pools["stats"]
        pre = pools["pre"]
        psum_tr = pools["psum_tr"]
        psum_acc = pools["psum_acc"]
        psum_pv = pools["psum_pv"]

        # ---- weight/const DMAs ------------------------------------------
        # DMAQ: which queues carry DMAs. Real-HW probe showed a single
        # queue reaches full bandwidth while SWDGE (Pool) DMAs are slow.
        dmaq_mode = os.environ.get("DMAQ", "one")
        qs = {"one": [nc.sync],
              "wact": [nc.sync],
              "two": [nc.sync, nc.scalar],
              "three": [nc.sync, nc.scalar, nc.gpsimd]}[dmaq_mode]
        nq = len(qs)
        # weight queue: in "wact" mode the weights ride the Act queue,
        # freeing the SP queue for the kv stream.
        wq_eng = nc.scalar if dmaq_mode == "wact" else qs[0]
        wqT = pre.tile([128, DC2, 2, D], E4, bufs=2, name="wqT_sb")
        wk = pre.tile([128, JC2, 2, D], E4, bufs=2, name="wk_sb")
        if nq == 1:
            dma32(wq_eng, wqT[:], wqT_d)
            dma32(wq_eng, wk[:], wk_d)
        else:
            dma32(qs[0], wqT[:, 0:2], wqT_d[:, 0:2])
            dma32(qs[nq - 1], wqT[:, 2:4], wqT_d[:, 2:4])
            dma32(qs[(nq - 1) // 2], wk[:, 0:2], wk_d[:, 0:2])
            dma32(qs[0], wk[:, 2:4], wk_d[:, 2:4])
        # (wvT/woT DMAs are issued mid-stream: they are only needed by
        # the tail projections.)
        wvT = const.tile([128, DC, D], BF16, bufs=2, name="wvT")
        woT = const.tile([128, JC, D], BF16, bufs=2, name="woT")

        qT = pre.tile([128, DC2, 2, B], E4, bufs=2, name="qT_sb")
        dma32(nc.sync, qT[:], qT_d)

        # rT[p, dc2, i, b, h] = r[b, h, (dc2*2+i)*128+p]  (incl 1/8 scale)
        rT = const.tile([128, DC2, 2, B, H], E4, bufs=2, name="rT")
        # zTV[p, dc, b, c]: c<16 -> P[b, c, dc*128+p]; c=32 -> V[b, .]
        # (v-stationary P matmuls emit P and V already transposed)
        zTV = const.tile([128, DC, B, R17], BF16, bufs=2, name="zTV")
        nl128 = const.tile([128, JC, B], FP32, bufs=2, name="nl128")
        VWv = const.tile([128, JC, B], FP32, bufs=2, name="VWv")
        prod = const.tile([128, JC, B], FP32, bufs=2, name="prod")
        attnT = const.tile([128, JC, B], BF16, bufs=2, name="attnT")
        out_sb = const.tile([B, D], FP32, bufs=2, name="out_sb")

        # ---- preamble: qp, r --------------------------------------------
        # qpT[p, jc, b] = qp[b, jc*128+p]
        qpT = pre.tile([128, JC, B], FP32, bufs=2, name="qpT")
        for jc in range(JC):
            ps = psum_acc.tile([128, B], FP32, tag="acc", name=f"ps_qp{jc}")
            for dc2 in range(DC2):
                nc.tensor.matmul(
                    ps[:], wqT[:, dc2, :, jc * 128:(jc + 1) * 128],
                    qT[:, dc2], start=(dc2 == 0), stop=(dc2 == DC2 - 1),
                    perf_mode=DR)
            nc.vector.tensor_copy(qpT[:, jc, :], ps[:])

        # Q[p, jc, b, h] = qp[b, jc*128+p]/8 if head(jc*128+p)==h else 0
        Q = pre.tile([128, JC, B, H], E4, bufs=2, name="Q")
        nc.vector.tensor_tensor(
            Q[:],
            qpT[:, :, :, None].to_broadcast((128, JC, B, H)),
            mask_sb[:, :, None, :].to_broadcast((128, JC, B, H)),
            mybir.AluOpType.mult)

        # rT[d, (b,h)] = sum_j Wk[j, d] * Q[j, (b,h)]   (DoubleRow: 2 jc
        # chunks per instruction)
        for dc in range(DC):
            ps = psum_acc.tile([128, B * H], FP32, tag="acc", name=f"ps_r{dc}")
            for jc2 in range(JC2):
                nc.tensor.matmul(
                    ps[:], wk[:, jc2, :, dc * 128:(dc + 1) * 128],
                    Q[:, 2 * jc2:2 * jc2 + 2, :, :],
                    start=(jc2 == 0), stop=(jc2 == JC2 - 1), perf_mode=DR)
            nc.vector.tensor_copy(rT[:, dc // 2, dc % 2, :, :], ps[:])

        # ---- main loop: stream keys/values ------------------------------
        # v-stationary P/V accumulation: psP[d, c] = sum_s v[s,d]*dotT[s,c]
        # (33-wide moving side; v rides pipelined ldweights; result IS the
        # transposed z layout, V in column 32). dc-outer so psum
        # accumulation groups are sequential (one bank zero-region each).
        NSP = NSB * NST // 2

        def p_burst(bb, v_ts, d17_ts, dcs):
            for dc in dcs:
                psP = psum_pv.tile([128, R17], FP32, tag="pv",
                                   name=f"pv_{bb}_{dc}")
                for sp in range(NSP):
                    sts = slice((2 * sp) % NST, (2 * sp) % NST + 2)
                    nc.tensor.matmul(
                        psP[:],
                        v_ts[sp // 2][:, sts, dc * 128:(dc + 1) * 128],
                        d17_ts[sp // 2][:, sts, :],
                        start=(sp == 0), stop=(sp == NSP - 1),
                        perf_mode=DR)
                nc.vector.tensor_copy(zTV[:, dc, bb, :], psP[:])

        pending = []
        S_all = stats.tile([16, B], FP32, tag="S", bufs=2, name="S_all")
        for b in range(B):
            dot_b = dot_tiles[b % 2]
            v_tiles = []
            d17_tiles = []
            scratch = stats.tile([16, 512], FP32, tag="scratch", bufs=2,
                                 name=f"scr{b}")
            S_parts = stats.tile([16, 4], FP32, tag="Sp", bufs=2,
                                 name=f"Sp{b}")
            for sblk in range(NSB):
                # kv slabs round-robin over the DMA queues in use
                i = NSB * b + sblk
                kT = kp.tile([128, DC2, 2, SBLK], E4, tag="kT",
                             name=f"kT_{b}_{sblk}")
                dma32(qs[i % nq], kT[:], kT_d[b, sblk])
                v_t = vp.tile([128, NST, D], E4, tag="v",
                              name=f"v_{b}_{sblk}")
                dma32(qs[(i + 1) % nq], v_t[:], v_d[b, sblk])
                v_tiles.append(v_t)
                if sblk == 0 and b == 1:
                    (wq_eng if nq == 1 else qs[nq - 1]).dma_start(wvT[:], wvT_d)
                if sblk == 0 and b == 2:
                    (wq_eng if nq == 1 else qs[nq - 1]).dma_start(woT[:], woT_d)
                # dot[h, s'] (shifted by LOGIT_SHIFT) for this slab
                for blk in range(NBLK if ABL != "dot" else 0):
                    s0 = sblk * SBLK + blk * 512
                    ps_dot = psum_acc.tile([16, 512], FP32, tag="acc",
                                           name=f"ps_dot{b}_{sblk}_{blk}")
                    for dc2 in range(DC2):
                        nc.tensor.matmul(
                            ps_dot[:], rT[:, dc2, :, b, :],
                            kT[:, dc2, :, blk * 512:(blk + 1) * 512],
                            start=(dc2 == 0), stop=(dc2 == DC2 - 1),
                            perf_mode=DR)
                    nc.vector.tensor_scalar_add(
                        dot_b[0:16, s0:s0 + 512], ps_dot[:], LOGIT_SHIFT)
                    # partial softmax denominator for this 512-block:
                    # dot_b is dot-4 (|dot| <~ 6) so exp cannot overflow
                    # and no running-max subtraction is needed.
                    nc.scalar.activation(
                        scratch[:, 0:512], dot_b[0:16, s0:s0 + 512], Exp,
                        bias=0.0, scale=1.0,
                        accum_out=S_parts[:, sblk * NBLK + blk:
                                          sblk * NBLK + blk + 1])
                # dotT tiles (4 transposes -> one drain); then
                # v-stationary P/V accumulation: psP[dc][d, c] +=
                # sum_s v[s, d] * dotT[s, c] - the 33-wide moving side
                # rides the PE cheaply, v loads on the ldweights path,
                # and the psum IS the transposed z layout (V in col 32).
                ps_t = psum_tr.tile([128, NST, R17], FP32, tag="tr",
                                    name=f"ps_dt{b}_{sblk}")
                for st in range(NST):
                    cols = slice(sblk * SBLK + st * 128,
                                 sblk * SBLK + (st + 1) * 128)
                    nc.tensor.transpose(ps_t[:, st, :], dot_b[:, cols],
                                        ident[:])
                d17 = d17p.tile([128, NST, R17], E4, tag="d17",
                                name=f"d17_{b}_{sblk}")
                nc.vector.tensor_copy(d17[:], ps_t[:])
                d17_tiles.append(d17)
                # interleave 2 dc-chunks of the previous batch's P burst
                if pending:
                    pb, pv_ts, pd_ts = pending[0]
                    p_burst(pb, pv_ts, pd_ts, [2 * sblk, 2 * sblk + 1])
                    if sblk == NSB - 1:
                        pending.pop(0)
            # ---- per-b: finish softmax denominator ----------------------
            nc.vector.reduce_sum(S_all[:, b:b + 1], S_parts[:], axis=X)
            # ---- stash b's v/d17 tiles; P burst is emitted interleaved
            # into b+1's slab loop (see p_burst) so the PE work spreads
            # across the DMA stream instead of lumping at the boundary.
            pending.append((b, v_tiles, d17_tiles))

        # flush the last batch's P burst
        pb, pv_ts, pd_ts = pending.pop(0)
        p_burst(pb, pv_ts, pd_ts, range(DC))

        # ---- tail ------------------------------------------------------
        # lse: one Ln pass; scatter -lse[b,h] to the attnT row layout.
        lnS = stats.tile([16, B], FP32, tag="lnS", name="lnS")
        nc.scalar.activation(lnS[:], S_all[:], Ln)
        neg_lse = stats.tile([16, B], FP32, tag="neg_lse", name="nlse")
        nc.vector.tensor_scalar_mul(neg_lse[:], lnS[:], -1.0)
        for jc in range(JC if ABL != "tail" else 0):
            ps_n = psum_tr.tile([128, B], FP32, tag="tr", name=f"ps_nl{jc}")
            nc.tensor.matmul(ps_n[:], maskT[:, jc, :], neg_lse[:],
                             start=True, stop=True)
            nc.vector.tensor_copy(nl128[:, jc, :], ps_n[:])
        # VWv[p, jc, b] = V[b, :] . Wv[jc*128+p, :]
        for jc in range(JC if ABL != "tail" else 0):
            ps_v = psum_acc.tile([128, B], FP32, tag="acc", name=f"ps_vw{jc}")
            for dc in range(DC):
                nc.tensor.matmul(
                    ps_v[:], wvT[:, dc, jc * 128:(jc + 1) * 128],
                    zTV[:, dc, :, 32], start=(dc == 0), stop=(dc == DC - 1))
            nc.vector.tensor_copy(VWv[:, jc, :], ps_v[:])
        # prod = (-lse) * VWv, then attn = P.Wv + prod fused into the
        # PSUM-drain copies of the P.Wv matmuls. The out-projection
        # accumulation is interleaved per jc so PE overlaps it with the
        # remaining attn groups instead of running two serial phases.
        nc.vector.tensor_tensor(prod[:], nl128[:], VWv[:],
                                mybir.AluOpType.mult)
        ps_o0 = psum_tr.tile([B, 512], FP32, tag="tr", name="ps_o0")
        ps_o1 = psum_tr.tile([B, 512], FP32, tag="tr", name="ps_o1")
        for jc in range(JC if ABL != "tail" else 0):
            ps_a = psum_acc.tile([128, B, 2], FP32, tag="acc",
                                 name=f"ps_a{jc}")
            for dc in range(DC):
                nc.tensor.matmul(
                    ps_a[:], wvT[:, dc, jc * 128:(jc + 1) * 128],
                    zTV[:, dc, :, 2 * jc:2 * jc + 2],
                    start=(dc == 0), stop=(dc == DC - 1))
            nc.vector.tensor_tensor(attnT[0:64, jc, :], ps_a[0:64, :, 0],
                                    prod[0:64, jc, :],
                                    mybir.AluOpType.add)
            nc.vector.tensor_tensor(attnT[64:128, jc, :], ps_a[64:128, :, 1],
                                    prod[64:128, jc, :],
                                    mybir.AluOpType.add)
            nc.tensor.matmul(ps_o0[:], attnT[:, jc, :], woT[:, jc, 0:512],
                             start=(jc == 0), stop=(jc == JC - 1))
            nc.tensor.matmul(ps_o1[:], attnT[:, jc, :], woT[:, jc, 512:1024],
                             start=(jc == 0), stop=(jc == JC - 1))
        nc.vector.tensor_copy(out_sb[:, 0:512], ps_o0[:])
        nc.gpsimd.dma_start(out_d[:, 0:512], out_sb[:, 0:512])
        nc.vector.tensor_copy(out_sb[:, 512:1024], ps_o1[:])
        nc.gpsimd.dma_start(out_d[:, 512:1024], out_sb[:, 512:1024])


_NC_CACHE = {}


def get_program():
    if "nc" not in _NC_CACHE:
        _NC_CACHE["nc"] = build_program()
    return _NC_CACHE["nc"]


def _dither_quant(x, npdt):
    """Error-diffusion quantization along axis 0 (preserves column sums
    to within one quantization step)."""
    x = np.asarray(x, dtype=np.float32)
    out = np.empty(x.shape, dtype=npdt)
    e = np.zeros(x.shape[1:], dtype=np.float32)
    for s in range(x.shape[0]):
        t = x[s] + e
        q = t.astype(npdt)
        out[s] = q
        e = t - q.astype(np.float32)
    return out


def make_in_maps(query, keys, values, Wq, Wk, Wv, Wo):
    """Host-side shard + layout packing (permutation / dtype cast)."""
    keys = np.asarray(keys, dtype=np.float32).astype(NP_E4)
    values = _dither_quant(values, NP_E4)

    def permW(W, npdt):  # [p, dc, j] = W[j, dc*128+p]
        W = np.asarray(W, dtype=np.float32).astype(npdt)
        return np.ascontiguousarray(
            W.T.reshape(DC, 128, D).transpose(1, 0, 2))

    def permW2(W, npdt):  # [p, dc2, i, j] = W[j, (dc2*2+i)*128+p]
        W = np.asarray(W, dtype=np.float32).astype(npdt)
        return np.ascontiguousarray(
            W.T.reshape(DC2, 2, 128, D).transpose(2, 0, 1, 3))

    def natW2(W, npdt):  # [p, jc2, i, d] = W[(jc2*2+i)*128+p, d]
        W = np.asarray(W, dtype=np.float32).astype(npdt)
        return np.ascontiguousarray(
            W.reshape(JC2, 2, 128, D).transpose(2, 0, 1, 3))

    wqT = permW2(Wq, NP_E4)
    wk = natW2(Wk, NP_E4)
    wvT = permW(Wv, NP_BF16)
    woT = permW(Wo, NP_BF16)

    # kT[b, sblk, p, dc2, i, s'] = keys[sblk*SBLK+s', b, (dc2*2+i)*128+p]
    kT_all = keys.transpose(1, 2, 0).reshape(BS, DC2, 2, 128, NSB, SBLK)
    kT_all = kT_all.transpose(0, 4, 3, 1, 2, 5)
    # vv[b, sblk, p, st, d] = values[sblk*SBLK+st*128+p, b, d]
    v_all = values.reshape(NSB, NST, 128, BS, D).transpose(3, 0, 2, 1, 4)

    query = np.asarray(query, dtype=np.float32).astype(NP_E4)
    in_maps = []
    for i in range(NCORES):
        sl = slice(B * i, B * (i + 1))
        qT = np.ascontiguousarray(
            query[sl].T.reshape(DC2, 2, 128, B).transpose(2, 0, 1, 3))
        in_maps.append({
            "qT": qT,
            "kT": np.ascontiguousarray(kT_all[sl]),
            "vv": np.ascontiguousarray(v_all[sl]),
            "wqT": wqT, "wk": wk, "wvT": wvT, "woT": woT,
        })
    return in_maps


def kernel(query, keys, values, Wq, Wk, Wv, Wo):
    nc = get_program()
    in_maps = make_in_maps(query, keys, values, Wq, Wk, Wv, Wo)
    res = bass_utils.run_bass_kernel_spmd(nc, in_maps, core_ids=list(range(NCORES)))
    return np.concatenate(
        [res.results[i]["out"] for i in range(NCORES)], axis=0)
